# revision 1
# baseline (speedup 1.0000x reference)
"""Trainium2 Bass kernel for hierarchical 2-layer GAT (nn_GAT_20383914787079).

Data-parallel over 8 NeuronCores: each core owns B/8 = 128 root nodes and
their full neighbor subtree (1280 level-1 rows, 32000 level-2 rows).

Restructured GAT (mathematically identical to the reference):
  per head h:  agg_h = A_h @ x_neigh   (A_h = block-diag softmax alphas)
               g[:, h*D:(h+1)*D] = agg_h @ W_h
  attention logits via precombined vectors u = W@a_self, v = W@a_neigh:
               es = x_self @ u,  en = x_neigh @ v
So the expensive projection of all 32000 neighbor rows is replaced by
projecting the 1280 aggregated rows; the h2 stream feeds only (a) a
transpose+tiny-matmul for `en` and (b) the alpha-weighted aggregation.

All large tensors are bf16 on-chip (fp32 HBM reads, cast during DMA).
"""
import numpy as np
import ml_dtypes

import concourse.bass as bass
import concourse.tile as tile
from concourse import mybir
from concourse.vector_clock import ScopedClock
from concourse.bass_utils import run_bass_kernel_spmd

BF = mybir.dt.bfloat16
F32 = mybir.dt.float32
NPBF = ml_dtypes.bfloat16

NEG = 0.2
NCORES = 8
B, FD, D, H, OUT = 128, 128, 128, 2, 128   # per-core roots, dims
R0, R1 = 10, 25
M1 = B * R0                                 # 1280 level-1 rows per core
M2 = M1 * R1                                # 32000 level-2 rows per core

# tiling
T2, G2, NC2 = 100, 4, 320      # j1: k=25 tiles
T1, G1, NC1 = 80, 8, 16        # j0/L1: k=10 tiles
NBLK, CPB = 5, 64              # j1 processed in 5 blocks of 64 chunks


def _install_drain_patch():
    """This container's walrus rejects >1 sync-wait per instruction; split the
    Tile tail-drain waits across SP NoOps."""
    def _patched(self, tick_clock, wait_clock):
        nc = self.nc
        probe = nc.sync.nop(nofuse=True, hint="drain_wait_split")
        wait_clock.add_sem_waits(probe.ins,
                                 ScopedClock({None: tick_clock.global_clock}))
        si = probe.ins.sync_info
        waits = list(si.on_wait) if si is not None and si.on_wait else []
        if len(waits) > 1:
            si.on_wait = [waits[0]]
            for wi in waits[1:]:
                n2 = nc.sync.nop(nofuse=True, hint="drain_wait_split")
                if n2.ins.sync_info is None:
                    n2.ins.sync_info = mybir.SyncInfo(on_wait=[wi], on_update=[])
                else:
                    n2.ins.sync_info.on_wait = [wi]
        nc.sync.drain()
        nc.all_engine_barrier()
        popped = nc._tile_sem_poison_stack.pop()
        assert popped is self._sem_poison
        nc.clear_and_free_semaphores(list(self.sems.allocated().values()))
        nc.all_engine_barrier()

    tile.TileContext._drain_and_barrier = _patched


def _split_multi_waits(nc):
    """Walrus here allows only one sync-wait per instruction: hoist extra
    waits onto same-engine NoOps inserted immediately before."""
    nid = [0]
    for fn in nc.m.functions:
        for bb in fn.blocks:
            insts = bb.instructions
            i = 0
            while i < len(insts):
                inst = insts[i]
                si = inst.sync_info
                if si is not None and si.on_wait and len(si.on_wait) > 1:
                    waits = list(si.on_wait)
                    si.on_wait = [waits[-1]]
                    for w in waits[:-1]:
                        nid[0] += 1
                        nop = mybir.InstNoOp(
                            name=f"waitsplit-{nid[0]}", ins=[], outs=[],
                            sync_info=mybir.SyncInfo(on_wait=[w], on_update=[]))
                        nop.engine = inst.engine
                        insts.insert(i, nop)
                        i += 1
                i += 1


def host_params(W0, a_s0, a_n0, W1, a_s1, a_n1, fc_W):
    """Small parameter prep on host (fp64 for the tiny contractions).
    All params are packed into two [128, N] blobs (one bf16, one fp32) so the
    device needs just two parameter DMAs."""
    W0c = np.transpose(np.float64(W0), (1, 0, 2)).reshape(FD, H * D)
    W1c = np.transpose(np.float64(W1), (1, 0, 2)).reshape(H * D, H * D)
    u0 = np.einsum("hfd,hd->fh", np.float64(W0), np.float64(a_s0))
    v0 = np.einsum("hfd,hd->fh", np.float64(W0), np.float64(a_n0))
    u1 = np.einsum("hcd,hd->ch", np.float64(W1), np.float64(a_s1))
    v1 = np.einsum("hcd,hd->ch", np.float64(W1), np.float64(a_n1))
    fcW = np.float64(fc_W)
    eye = np.eye(128)
    masks = {}
    for K, T in ((25, T2), (10, T1)):
        G = T // K
        m = (np.arange(T)[:, None] // K == np.arange(G)[None, :]).astype(np.float64)
        mp = np.zeros((128, G)); mp[:T] = m
        mep = np.zeros((128, T)); mep[:G] = m.T
        masks[K] = (mp, mep)

    def pad(a):
        out = np.zeros((128, a.shape[1]))
        out[:a.shape[0]] = a
        return out

    bf_parts = [W0c, W1c[:128], W1c[128:],
                np.concatenate([u1[:128], u1[128:]], axis=1),
                np.concatenate([v1[:128], v1[128:]], axis=1),
                fcW[:128], fcW[128:], eye, pad(masks[10][0][:T1])]
    f32_parts = [eye, u0, v0, masks[25][0], masks[25][1], masks[25][0],
                 masks[10][0], masks[10][1], masks[10][0]]
    pbf = np.concatenate(bf_parts, axis=1).astype(NPBF)
    pf32 = np.concatenate(f32_parts, axis=1).astype(np.float32)
    return {"pbf": np.ascontiguousarray(pbf),
            "pf32": np.ascontiguousarray(pf32)}


# column offsets in the blobs
BF_COLS = {"W0c": (0, 256), "W1ca": (256, 512), "W1cb": (512, 768),
           "u1": (768, 772), "v1": (772, 776), "fcWa": (776, 904),
           "fcWb": (904, 1032), "identb": (1032, 1160), "stair10b": (1160, 1168)}
F32_COLS = {"identf": (0, 128), "u0": (128, 130), "v0": (130, 132),
            "mask25": (132, 136), "maskE25": (136, 236), "stair25f": (236, 240),
            "mask10": (240, 248), "maskE10": (248, 328), "stair10f": (328, 336)}
BF_N = 1168
F32_N = 336


def build_program(split_waits=True):
    nc = bass.Bass()
    dp = nc.declare_dram_parameter
    h0 = dp("h0", [B, FD], F32, isOutput=False)
    h1 = dp("h1", [M1, FD], F32, isOutput=False)
    h2 = dp("h2", [M2, FD], F32, isOutput=False)
    y = dp("y", [B, OUT], F32, isOutput=True)
    pbf_d = dp("pbf", [128, BF_N], BF, isOutput=False)
    pf32_d = dp("pf32", [128, F32_N], F32, isOutput=False)

    copy_ctr = [0]

    with tile.TileContext(nc) as tc:
        with (tc.tile_pool(name="prm", bufs=1) as prm,
              tc.tile_pool(name="big", bufs=1) as big,
              tc.tile_pool(name="h2p", bufs=3) as h2p,
              tc.tile_pool(name="x2t", bufs=3) as x2tp,
              tc.tile_pool(name="work", bufs=3) as wk,
              tc.tile_pool(name="pt", bufs=2, space="PSUM") as ppt,
              tc.tile_pool(name="pen", bufs=1, space="PSUM") as ppen,
              tc.tile_pool(name="pagg", bufs=2, space="PSUM") as ppagg,
              tc.tile_pool(name="psm", bufs=2, space="PSUM") as ppsm):

            def cp(dst, src):
                """PSUM->SBUF (or sbuf) copy, alternating DVE/ACT."""
                copy_ctr[0] += 1
                if copy_ctr[0] % 2:
                    nc.vector.tensor_copy(dst, src)
                else:
                    nc.scalar.copy(dst, src)

            # ---- params to SBUF (two blob DMAs) ----
            pbf = prm.tile([128, BF_N], BF, tag="pbf")
            nc.sync.dma_start(pbf[:], pbf_d[:])
            pf32 = prm.tile([128, F32_N], F32, tag="pf32")
            nc.sync.dma_start(pf32[:], pf32_d[:])
            S = {}
            for nm, (c0, c1) in BF_COLS.items():
                S[nm] = pbf[:, c0:c1]
            for nm, (c0, c1) in F32_COLS.items():
                S[nm] = pf32[:, c0:c1]
            identb, identf = S["identb"], S["identf"]
            mask25 = S["mask25"][:T2, :]
            maskE25 = S["maskE25"][:G2, :]
            stair25 = S["stair25f"][:T2, :]
            mask10 = S["mask10"][:T1, :]
            maskE10 = S["maskE10"][:G1, :]
            stair10f = S["stair10f"][:T1, :]
            stair10b = S["stair10b"][:T1, :]

            # ---- h2 block 0 first (keeps the DMA device busy from t=0),
            # then h1/h0, then the remaining h2 blocks ----
            BLOCKS = [64, 64, 64, 48, 40, 24, 16]   # sums to NC2; 16-aligned
            hbs, c0s = [], []
            h2v = h2.rearrange("(c p) f -> p c f", p=T2)
            _off = [0]

            def issue_hb():
                b = len(hbs)
                bn, c0 = BLOCKS[b], _off[0]
                hb = h2p.tile([T2, bn * FD], F32, tag="h2blk", name="hb")
                nc.sync.dma_start(
                    hb[:].rearrange("p (c f) -> p c f", f=FD),
                    h2v[:, c0:c0 + bn, :])
                hbs.append(hb)
                c0s.append(c0)
                _off[0] += bn

            # ---- h0 / h1 loads (fp32, HWDGE) ----
            h1row = big.tile([T1, NC1 * FD], F32, tag="h1row")     # [80, 2048]
            nc.sync.dma_start(
                h1row[:].rearrange("p (c f) -> p c f", f=FD),
                h1.rearrange("(c p) f -> p c f", p=T1))
            h0row = big.tile([128, FD], F32, tag="h0row")
            nc.sync.dma_start(h0row[:], h0[:])
            for _ in range(len(BLOCKS)):
                issue_hb()

            # ---- X1T: transpose h1 tiles -> [128, 1280] fp32 ----
            x1t = big.tile([128, M1], F32, tag="x1t")
            for grp in range(4):      # 4 groups of 4 tiles -> psum [128, 320]
                pt = ppt.tile([128, 4 * T1], F32, tag="pt")
                for j in range(4):
                    cidx = 4 * grp + j
                    nc.tensor.transpose(pt[:, j * T1:(j + 1) * T1],
                                        h1row[:, cidx * FD:(cidx + 1) * FD],
                                        identf[:T1, :T1])
                cp(x1t[:, grp * 4 * T1:(grp + 1) * 4 * T1], pt[:])

            # h0T
            pt = ppt.tile([128, 128], F32, tag="pt")
            nc.tensor.transpose(pt[:], h0row[:], identf[:])
            h0t = big.tile([128, 128], F32, tag="h0t")
            cp(h0t[:], pt[:])

            # ---- es1 (j1 self): lhsT=u0, rhs=X1T strips -> es1T [2, 1280] ----
            es1T = big.tile([2, M1], F32, tag="es1T")
            for w, (c0, c1) in enumerate(((0, 6), (6, 12), (12, 16))):
                psw = ppsm.tile([2, 480], F32, tag="psm")
                for cc in range(c0, c1):
                    nc.tensor.matmul(psw[:, (cc - c0) * T1:(cc - c0 + 1) * T1],
                                     S["u0"][:], x1t[:, cc * T1:(cc + 1) * T1])
                cp(es1T[:, c0 * T1:c1 * T1], psw[:, :(c1 - c0) * T1])
            # es1_arr[g', NC2*h + c] = es1[m=4c+g', h]
            es1_arr = big.tile([G2, 2 * NC2], F32, tag="es1_arr")
            for g in range(G2):
                for h in range(H):
                    nc.gpsimd.dma_start(
                        es1_arr[g:g + 1, NC2 * h:NC2 * (h + 1)],
                        es1T[h:h + 1, :].rearrange("o (c g) -> o c g", g=G2)[:, :, g])

            # ---- es0 (j0 self): lhsT=u0, rhs=h0T -> es0T [2, 128] ----
            ps0 = ppsm.tile([2, 128], F32, tag="psm")
            nc.tensor.matmul(ps0[:], S["u0"][:], h0t[:])
            es0T = big.tile([2, 128], F32, tag="es0T")
            cp(es0T[:], ps0[:])
            es0_arr = big.tile([G1, 2 * NC1], F32, tag="es0_arr")
            for g in range(G1):
                for h in range(H):
                    nc.gpsimd.dma_start(
                        es0_arr[g:g + 1, NC1 * h:NC1 * (h + 1)],
                        es0T[h:h + 1, :].rearrange("o (c g) -> o c g", g=G1)[:, :, g])

            # ---- en1 (j0 neigh): lhsT=X1T tile, rhs=v0 -> [80, 2] windows ----
            pen1 = ppen.tile([T1, 2 * NC1], F32, tag="pen")
            for cc in range(NC1):
                nc.tensor.matmul(pen1[:, 2 * cc:2 * cc + 2],
                                 x1t[:, cc * T1:(cc + 1) * T1], S["v0"][:])
            en1 = big.tile([T1, 2 * NC1], F32, tag="en1")
            cp(en1[:], pen1[:])

            # ============ shared softmax+agg machinery ============
            def softmax_alpha(T, G, K, NCc, en_sb, es_arr_slice, mask, maskE,
                              stair, adt, blk_tag):
                """en_sb [T, 2*NCc] fp32 sbuf; es_arr_slice [G, NCc, H] fp32 AP.
                Returns astrip [T, NCc*2*G] in dtype adt."""
                W = 2 * NCc
                pE = ppsm.tile([T, W], F32, tag="psm")          # esE expand
                nc.tensor.matmul(pE[:], maskE[:], es_arr_slice)
                e1 = wk.tile([T, W], F32, tag=f"e1{blk_tag}")
                nc.vector.tensor_add(e1[:], en_sb, pE[:])
                e2 = wk.tile([T, W], F32, tag=f"e2{blk_tag}")
                nc.vector.scalar_tensor_tensor(e2[:], e1[:], NEG, e1[:],
                                               mybir.AluOpType.mult,
                                               mybir.AluOpType.max)
                ex = wk.tile([T, W], F32, tag=f"ex{blk_tag}")
                nc.scalar.activation(ex[:], e2[:], mybir.ActivationFunctionType.Exp)
                pden = ppsm.tile([G, W], F32, tag="psm")
                nc.tensor.matmul(pden[:], mask[:], ex[:])
                den = wk.tile([G, W], F32, tag=f"den{blk_tag}")
                cp(den[:], pden[:])
                rden = wk.tile([G, W], F32, tag=f"rden{blk_tag}")
                nc.vector.reciprocal(rden[:], den[:])
                pdE = ppsm.tile([T, W], F32, tag="psm")
                nc.tensor.matmul(pdE[:], maskE[:], rden[:])
                alpha = wk.tile([T, W], adt, tag=f"al{blk_tag}")
                nc.vector.tensor_mul(alpha[:], ex[:], pdE[:])
                astrip = wk.tile([T, NCc * 2 * G], adt, tag=f"as{blk_tag}")
                a4 = alpha[:].rearrange("p (c h) -> p c h", h=H)
                a4 = a4.unsqueeze(3).broadcast_to([T, NCc, H, G])
                s4 = stair.unsqueeze(1).unsqueeze(1).broadcast_to([T, NCc, H, G])
                nc.vector.tensor_mul(
                    astrip[:].rearrange("p (c h g) -> p c h g", h=H, g=G), a4, s4)
                return astrip

            # ============ j0: agg over h1 (output transposed: [f, m-cols]) ====
            es0_v = es0_arr[:].rearrange("g (h c) -> g c h", h=H)
            astrip0 = softmax_alpha(T1, G1, R0, NC1, en1[:], es0_v,
                                    mask10, maskE10, stair10f, F32, "j0")
            # aggT0[f, 16*cc + 8h + g] = sum_p h1row[p, cc*FD+f]*astrip0[p, ...]
            pgj0 = ppagg.tile([128, 2 * G1 * NC1], F32, tag="paggs", bufs=1)
            for cc in range(NC1):
                nc.tensor.matmul(pgj0[:, 16 * cc:16 * cc + 16],
                                 h1row[:, cc * FD:(cc + 1) * FD],
                                 astrip0[:, 16 * cc:16 * cc + 16])
            aggT0 = big.tile([128, 2 * G1 * NC1], BF, tag="aggT0")
            cp(aggT0[:], pgj0[:])

            # j0 projection -> g0t [128 d, (h, m0)], m0 = 8cc + g
            g0t = big.tile([128, 2 * B], BF, tag="g0t")
            pj0 = ppagg.tile([128, 2 * B], F32, tag="paggs", bufs=1)
            for h in range(H):
                rhs = aggT0[:].rearrange("p (cc h g) -> p cc h g",
                                         h=H, g=G1)[:, :, h, :]
                nc.tensor.matmul(pj0[:, B * h:B * (h + 1)],
                                 S["W0c"][:, 128 * h:128 * (h + 1)], rhs)
            cp(g0t[:], pj0[:])

            # es_L1: lhsT = u1 chunks, rhs = g0T slabs -> [2, 128] accumulate
            psL = ppsm.tile([2, B], F32, tag="psm")
            for hp in range(H):
                nc.tensor.matmul(psL[:], S["u1"][:, 2 * hp:2 * hp + 2],
                                 g0t[:, B * hp:B * (hp + 1)],
                                 start=(hp == 0), stop=(hp == 1))
            esLT = big.tile([2, B], F32, tag="esLT")
            cp(esLT[:], psL[:])
            esL_arr = big.tile([G1, 2 * NC1], F32, tag="esL_arr")
            for g in range(G1):
                for h in range(H):
                    nc.gpsimd.dma_start(
                        esL_arr[g:g + 1, NC1 * h:NC1 * (h + 1)],
                        esLT[h:h + 1, :].rearrange("o (c g) -> o c g", g=G1)[:, :, g])

            # ============ j1: stream h2 in NBLK blocks (fp32 tiles) ============
            en_sb = big.tile([T2, 2 * NC2], F32, tag="en_sb")      # [100, 640]
            # aggT1 [128 f, (q, cl, h, g)]: col = 128q + 8cl + 4h + g
            aggT1 = big.tile([128, 20 * 128], BF, tag="aggT1")
            g1t = big.tile([128, 2 * M1], BF, tag="g1t")
            g1row = big.tile([T1, NC1 * 2 * FD], BF, tag="g1row")
            pagg_cur = [None]
            g1row_done = [0]

            def project_q(q):
                # pj [128 d, (h, mloc)] for m-window [64q, 64q+64)
                pj = ppagg.tile([128, 128], F32, tag="paggs", bufs=1,
                                name="pj_q")
                for h in range(H):
                    rhs = aggT1[:, q * 128:(q + 1) * 128].rearrange(
                        "p (cl h g) -> p cl h g", h=H, g=G2)[:, :, h, :]
                    nc.tensor.matmul(pj[:, 64 * h:64 * h + 64],
                                     S["W0c"][:, 128 * h:128 * (h + 1)], rhs)
                dst = g1t[:].rearrange("p (h q m) -> p h q m",
                                       h=H, q=20)[:, :, q, :]
                cp(dst, pj[:].rearrange("p (h m) -> p h m", h=H))
                # g1row transposes for completed 80-wide windows
                while 80 * (g1row_done[0] + 1) <= 64 * (q + 1):
                    t = g1row_done[0]
                    for hp in range(H):
                        pt5 = ppt.tile([T1, 128], BF, tag="pt", name="pt5")
                        nc.tensor.transpose(
                            pt5[:],
                            g1t[:, M1 * hp + T1 * t: M1 * hp + T1 * (t + 1)],
                            identb[:])
                        cp(g1row[:, (2 * t + hp) * FD:(2 * t + hp + 1) * FD],
                           pt5[:])
                    g1row_done[0] += 1

            def agg1_flush(q):
                cp(aggT1[:, q * 128:(q + 1) * 128], pagg_cur[0][:])
                pagg_cur[0] = None
                project_q(q)

            def stage1(b):
                # transposes (pack 4 per psum tile) + en-mms
                hb, bn, c0 = hbs[b], BLOCKS[b], c0s[b]
                penb = ppen.tile([T2, 2 * bn], F32, tag="pen", name="penb")
                for grp in range((bn + 3) // 4):
                    cls = list(range(4 * grp, min(4 * grp + 4, bn)))
                    pt3 = ppt.tile([128, len(cls) * T2], F32, tag="pt",
                                   name="pt3")
                    for j, cl in enumerate(cls):
                        nc.tensor.transpose(pt3[:, j * T2:(j + 1) * T2],
                                            hb[:, cl * FD:(cl + 1) * FD],
                                            identf[:T2, :T2])
                    xs = x2tp.tile([128, len(cls) * T2], F32, tag="x2t",
                                   name="xs")
                    cp(xs[:], pt3[:])
                    for j, cl in enumerate(cls):
                        nc.tensor.matmul(penb[:, 2 * cl:2 * cl + 2],
                                         xs[:, j * T2:(j + 1) * T2], S["v0"][:])
                cp(en_sb[:, 2 * c0:2 * (c0 + bn)], penb[:])

            def stage2(b):
                hb, bn, c0 = hbs[b], BLOCKS[b], c0s[b]
                es1_v = es1_arr[:].rearrange("g (h c) -> g c h", h=H)[
                    :, c0:c0 + bn, :]
                astr = softmax_alpha(T2, G2, R1, bn,
                                     en_sb[:, 2 * c0:2 * (c0 + bn)],
                                     es1_v, mask25, maskE25, stair25, F32, "j1")
                for cl in range(bn):
                    ci = c0 + cl
                    q, r = divmod(ci, 16)
                    if r == 0:
                        pagg_cur[0] = ppagg.tile([128, 128], F32, tag="pagg",
                                                 name="pagg_j1")
                    nc.tensor.matmul(pagg_cur[0][:, 8 * r:8 * r + 8],
                                     hb[:, cl * FD:(cl + 1) * FD],
                                     astr[:, 8 * cl:8 * cl + 8])
                    if r == 15:
                        agg1_flush(q)

            # 2-stage software pipeline: engines run in program order, so a
            # block's softmax chain must not sit ahead of the next block's
            # independent transposes in any engine queue.
            for b in range(len(BLOCKS)):
                stage1(b)
                if b > 0:
                    stage2(b - 1)
            stage2(len(BLOCKS) - 1)

            # ============ L1 ============
            # en_L1: lhsT = g1T slices, rhs = v1 chunk, accumulate chunks
            penL = ppen.tile([T1, 2 * NC1], F32, tag="pen")
            for t in range(NC1):
                for hp in range(H):
                    nc.tensor.matmul(penL[:, 2 * t:2 * t + 2],
                                     g1t[:, M1 * hp + T1 * t: M1 * hp + T1 * (t + 1)],
                                     S["v1"][:, 2 * hp:2 * hp + 2],
                                     start=(hp == 0), stop=(hp == 1))
            enL = big.tile([T1, 2 * NC1], F32, tag="enL")
            cp(enL[:], penL[:])

            esL_v = esL_arr[:].rearrange("g (h c) -> g c h", h=H)
            astrL = softmax_alpha(T1, G1, R0, NC1, enL[:], esL_v,
                                  mask10, maskE10, stair10b, BF, "L1")
            # aggT2 [128 (fp-slab d), (fp, t, h, g)]: col = 256fp + 16t + 8h + g
            aggT2 = big.tile([128, 2 * 256], BF, tag="aggT2")
            for fp in range(2):
                pg = ppagg.tile([128, 256], F32, tag="paggs", bufs=1)
                for t in range(NC1):
                    nc.tensor.matmul(pg[:, 16 * t:16 * t + 16],
                                     g1row[:, (2 * t + fp) * FD:(2 * t + fp + 1) * FD],
                                     astrL[:, 16 * t:16 * t + 16])
                cp(aggT2[:, 256 * fp:256 * (fp + 1)], pg[:])

            # L1 projection: ggt [128 d, (h, m0)], m0 = 8t + g
            ggt = big.tile([128, 2 * B], BF, tag="ggt")
            W1cs = (S["W1ca"], S["W1cb"])
            pjL = ppagg.tile([128, 2 * B], F32, tag="paggs", bufs=1)
            for h in range(H):
                for fp in range(2):
                    rhs = aggT2[:, 256 * fp:256 * (fp + 1)].rearrange(
                        "p (t h g) -> p t h g", h=H, g=G1)[:, :, h, :]
                    nc.tensor.matmul(pjL[:, B * h:B * (h + 1)],
                                     W1cs[fp][:, 128 * h:128 * (h + 1)], rhs,
                                     start=(fp == 0), stop=(fp == 1))
            cp(ggt[:], pjL[:])

            # fc: outT [o, m0] = sum_chunks fcW_chunk.T @ ggT_slab
            pfc = ppagg.tile([128, B], F32, tag="paggs", bufs=1)
            fcs = (S["fcWa"], S["fcWb"])
            for hp in range(H):
                nc.tensor.matmul(pfc[:], fcs[hp][:], ggt[:, B * hp:B * (hp + 1)],
                                 start=(hp == 0), stop=(hp == 1))
            outT = big.tile([128, B], F32, tag="outT")
            cp(outT[:], pfc[:])
            ptf = ppt.tile([128, B], F32, tag="pt")
            nc.tensor.transpose(ptf[:], outT[:], identf[:])
            outn = big.tile([B, OUT], F32, tag="outn")
            cp(outn[:], ptf[:])
            nc.sync.dma_start(y[:], outn[:])

    if split_waits:
        _split_multi_waits(nc)
    return nc


_PROG = None


def kernel(**inputs):
    global _PROG
    _install_drain_patch()
    P = host_params(inputs["W0"], inputs["a_self0"], inputs["a_neigh0"],
                    inputs["W1"], inputs["a_self1"], inputs["a_neigh1"],
                    inputs["fc_W"])
    if _PROG is None:
        _PROG = build_program()
    nc = _PROG
    h0 = np.ascontiguousarray(np.asarray(inputs["h0"], np.float32))
    h1 = np.ascontiguousarray(np.asarray(inputs["h1"], np.float32))
    h2 = np.ascontiguousarray(np.asarray(inputs["h2"], np.float32))
    in_maps = []
    for c in range(NCORES):
        m = {"h0": h0[B * c:B * (c + 1)],
             "h1": h1[M1 * c:M1 * (c + 1)],
             "h2": h2[M2 * c:M2 * (c + 1)]}
        m.update(P)
        in_maps.append(m)
    core_ids = list(range(NCORES))
    last = None
    for _attempt in range(3):
        try:
            res = run_bass_kernel_spmd(nc, in_maps, core_ids)
            out = np.concatenate([np.asarray(res.results[c]["y"])
                                  for c in core_ids], axis=0)
            return out.astype(np.float32)
        except Exception as e:   # transient device-unrecoverable happens
            last = e
    raise last



# revision 2
# speedup vs baseline: 2.3956x; 2.3956x over previous
"""Trainium2 Bass kernel for hierarchical 2-layer GAT (nn_GAT_20383914787079).

Data-parallel over 8 NeuronCores: each core owns B/8 = 128 root nodes and
their full neighbor subtree (1280 level-1 rows, 32000 level-2 rows).

Design notes:
  * All hierarchy inputs are packed to bf16 tile layouts on the host. The
    level-2 features are uploaded in BOTH layouts the kernel needs --
    k-on-partitions (for the alpha-weighted aggregation matmuls) and
    f-on-partitions (for the attention-logit matmuls) -- which removes every
    on-chip transpose/copy of the big stream. Total HBM bytes equal the
    original fp32 single-layout upload.
  * DMA is issued round-robin across the three DGE queues (SP / Activation
    HWDGE and the GPSIMD SWDGE path) so transfers overlap.
  * Every projection is folded into host-precomputed parameters:
      en_L1 = aggT1 @ (W0^T v1),  es_L1 = aggT0 @ (W0^T u1),
      y     = sum_{h',h} pre2_{h',h} @ H[h',h],  H = W0*W1*fcW folded,
    so the per-head GAT projections (g0/g1) are never materialized.
  * Softmax denominators are expanded with a single fused mask matmul
    (denE = M @ ex, M[t,p] = same-group indicator), halving the
    cross-engine hops in the per-block dependency chain.
  * Attention-logit rearranges (es -> grouped arrays) are done with tiny
    PE matmuls against 0/1 selector masks instead of descriptor-heavy DMAs.

j1 chunk remap: chunk ci (125 rows = 5 groups of 25 neighbors) owns level-1
rows m = 80*(ci//16) + 16*g + (ci%16); flush q = 16 chunks covers the
contiguous m-window [80q, 80q+80).

aggT1 column layout: col = 160q + 10r + 2g + h; the m-ordered per-head view
is [g stride 2][r stride 10], giving m = 80q + 16g + r.
"""
import numpy as np
import ml_dtypes

import concourse.bass as bass
import concourse.tile as tile
from concourse import mybir
from concourse.vector_clock import ScopedClock
from concourse.bass_utils import run_bass_kernel_spmd

BF = mybir.dt.bfloat16
F32 = mybir.dt.float32
NPBF = ml_dtypes.bfloat16

NEG = 0.2
NCORES = 8
B, FD, D, H, OUT = 128, 128, 128, 2, 128   # per-core roots, dims
R0, R1 = 10, 25
M1 = B * R0                                 # 1280
M2 = M1 * R1                                # 32000

T2, NC2, G2 = 125, 256, 5                   # j1 chunks
T1, NC1, G1 = 80, 16, 8                     # j0/L1 chunks
NQ = 16                                     # j1 flushes (80-m windows)
NBLK = 8                                    # h2 stream blocks (32 chunks each)
CPB = NC2 // NBLK                           # 32 chunks per block


def _install_drain_patch():
    """This container's walrus rejects >1 sync-wait per instruction; split the
    Tile tail-drain waits across SP NoOps."""
    def _patched(self, tick_clock, wait_clock):
        nc = self.nc
        probe = nc.sync.nop(nofuse=True, hint="drain_wait_split")
        wait_clock.add_sem_waits(probe.ins,
                                 ScopedClock({None: tick_clock.global_clock}))
        si = probe.ins.sync_info
        waits = list(si.on_wait) if si is not None and si.on_wait else []
        if len(waits) > 1:
            si.on_wait = [waits[0]]
            for wi in waits[1:]:
                n2 = nc.sync.nop(nofuse=True, hint="drain_wait_split")
                if n2.ins.sync_info is None:
                    n2.ins.sync_info = mybir.SyncInfo(on_wait=[wi], on_update=[])
                else:
                    n2.ins.sync_info.on_wait = [wi]
        nc.sync.drain()
        nc.all_engine_barrier()
        popped = nc._tile_sem_poison_stack.pop()
        assert popped is self._sem_poison
        nc.clear_and_free_semaphores(list(self.sems.allocated().values()))
        nc.all_engine_barrier()

    tile.TileContext._drain_and_barrier = _patched


def _split_multi_waits(nc):
    """Walrus here allows only one sync-wait per instruction: hoist extra
    waits onto same-engine NoOps inserted immediately before."""
    nid = [0]
    for fn in nc.m.functions:
        for bb in fn.blocks:
            insts = bb.instructions
            i = 0
            while i < len(insts):
                inst = insts[i]
                si = inst.sync_info
                if si is not None and si.on_wait and len(si.on_wait) > 1:
                    waits = list(si.on_wait)
                    si.on_wait = [waits[-1]]
                    for w in waits[:-1]:
                        nid[0] += 1
                        nop = mybir.InstNoOp(
                            name=f"waitsplit-{nid[0]}", ins=[], outs=[],
                            sync_info=mybir.SyncInfo(on_wait=[w], on_update=[]))
                        nop.engine = inst.engine
                        insts.insert(i, nop)
                        i += 1
                i += 1


# ---------------- host-side parameter folding + input packing --------------

def _cols():
    off = [0]
    d = {}

    def put(nm, n):
        d[nm] = (off[0], off[0] + n)
        off[0] += n
    put("u0", 2); put("v0", 2); put("Wu1", 4); put("Wv1", 4)
    put("H4", 512)
    put("maskE25", T2)       # [5,125]  expand es to rows
    put("M25", T2)           # [125,125] same-group indicator (denE)
    put("stair2", 2 * G2)    # [125,10] (t//25==g), (g,h) pairs
    put("stair10", 2 * G1)   # [80,16]  (t//10==g), (g,h) pairs
    put("maskE10", T1)       # [8,80]
    put("M10", T1)           # [80,80]
    put("ident", 128)
    put("sel5", G2)          # [80,5]  (t//16==g)
    put("Mr", 16)            # [80,16] (t%16==r)
    put("sel8", G1)          # [128,8] (m0%8==g)
    put("maskC16", NC1)      # [128,16] (m0//8==c)
    return d, off[0]


PCOLS, PN = _cols()


def host_params(W0, a_s0, a_n0, W1, a_s1, a_n1, fc_W):
    W0 = np.float64(W0); W1 = np.float64(W1)
    u0 = np.einsum("hfd,hd->fh", W0, np.float64(a_s0))
    v0 = np.einsum("hfd,hd->fh", W0, np.float64(a_n0))
    u1 = np.einsum("hcd,hd->ch", W1, np.float64(a_s1))
    v1 = np.einsum("hcd,hd->ch", W1, np.float64(a_n1))
    Wu1 = np.zeros((128, 4)); Wv1 = np.zeros((128, 4))
    for h in range(H):
        Wu1[:, 2 * h:2 * h + 2] = W0[h] @ u1[128 * h:128 * (h + 1), :]
        Wv1[:, 2 * h:2 * h + 2] = W0[h] @ v1[128 * h:128 * (h + 1), :]
    fcW = np.float64(fc_W)
    H4 = np.zeros((128, 512))          # col = (2h'+h)*128 + o
    for hp in range(H):
        Gp = W1[hp] @ fcW[128 * hp:128 * (hp + 1), :]
        for h in range(H):
            H4[:, (2 * hp + h) * 128:(2 * hp + h + 1) * 128] = \
                W0[h] @ Gp[128 * h:128 * (h + 1), :]

    blob = np.zeros((128, PN))

    def put(nm, a):
        c0, c1 = PCOLS[nm]
        blob[:a.shape[0], c0:c1] = a
    put("u0", u0); put("v0", v0); put("Wu1", Wu1); put("Wv1", Wv1)
    put("H4", H4)
    t2 = np.arange(T2); t1 = np.arange(T1); m0 = np.arange(128)
    m25 = (t2[:, None] // R1 == np.arange(G2)[None, :]).astype(float)
    put("maskE25", m25.T)
    put("M25", (t2[:, None] // R1 == t2[None, :] // R1).astype(float))
    put("stair2", np.repeat(m25, 2, axis=1))
    m10 = (t1[:, None] // R0 == np.arange(G1)[None, :]).astype(float)
    put("stair10", np.repeat(m10, 2, axis=1))
    put("maskE10", m10.T)
    put("M10", (t1[:, None] // R0 == t1[None, :] // R0).astype(float))
    put("ident", np.eye(128))
    put("sel5", (t1[:, None] // 16 == np.arange(G2)[None, :]).astype(float))
    put("Mr", (t1[:, None] % 16 == np.arange(16)[None, :]).astype(float))
    put("sel8", (m0[:, None] % 8 == np.arange(G1)[None, :]).astype(float))
    put("maskC16", (m0[:, None] // 8 == np.arange(NC1)[None, :]).astype(float))
    return np.ascontiguousarray(blob.astype(NPBF))


_CI = np.arange(NC2)
_MMAP = 80 * (_CI[:, None] // 16) + 16 * np.arange(G2)[None, :] + (_CI[:, None] % 16)


def pack_inputs(h0, h1, h2):
    """Full inputs -> per-core packed bf16 arrays (both h2 layouts)."""
    h0 = np.asarray(h0, np.float32); h1 = np.asarray(h1, np.float32)
    h2 = np.asarray(h2, np.float32)
    a2 = h2.reshape(NCORES, M1, R1, FD)
    pk = a2[:, _MMAP]                          # [8, 256, 5, 25, 128]
    pk = pk.transpose(0, 2, 3, 1, 4)           # [8, 5, 25, 256, 128]
    h2pk = np.ascontiguousarray(
        pk.reshape(NCORES, T2, NC2 * FD)).astype(NPBF)
    # f-on-partitions layout: col = 125*ci + p
    h2T = np.ascontiguousarray(
        pk.reshape(NCORES, T2, NC2, FD).transpose(0, 3, 2, 1)
        .reshape(NCORES, FD, NC2 * T2)).astype(NPBF)
    h1pk = np.ascontiguousarray(
        h1.reshape(NCORES, NC1, T1, FD).transpose(0, 2, 1, 3)
        .reshape(NCORES, T1, NC1 * FD)).astype(NPBF)
    h0pk = np.ascontiguousarray(h0.reshape(NCORES, B, FD)).astype(NPBF)
    return h0pk, h1pk, h2pk, h2T


# ----------------------------- device program ------------------------------

def build_program(split_waits=True):
    nc = bass.Bass()
    dp = nc.declare_dram_parameter
    h0d = dp("h0pk", [B, FD], BF, isOutput=False)
    h1d = dp("h1pk", [T1, NC1 * FD], BF, isOutput=False)
    h2d = dp("h2pk", [T2, NC2 * FD], BF, isOutput=False)
    h2Td = dp("h2T", [FD, NC2 * T2], BF, isOutput=False)
    pd = dp("prm", [128, PN], BF, isOutput=False)
    yd = dp("y", [B, OUT], F32, isOutput=True)

    with tile.TileContext(nc) as tc, \
         nc.allow_low_precision(reason="bf16 data path; 2e-2 tolerance"):
        with (tc.tile_pool(name="big", bufs=1) as big,
              tc.tile_pool(name="h2p", bufs=NBLK) as h2p,
              tc.tile_pool(name="h2tp", bufs=NBLK) as h2tp,
              tc.tile_pool(name="wk", bufs=3) as wk,
              tc.tile_pool(name="ppt", bufs=2, space="PSUM") as ppt,
              tc.tile_pool(name="ppen", bufs=2, space="PSUM") as ppen,
              tc.tile_pool(name="ppsm", bufs=1, space="PSUM") as ppsm,
              tc.tile_pool(name="ppagg", bufs=2, space="PSUM") as ppagg,
              tc.tile_pool(name="ppenL", bufs=1, space="PSUM") as ppenL):

            # ---- param / small-input DMAs (SP queue) ----
            prm = big.tile([128, PN], BF, tag="prm")
            nc.sync.dma_start(prm[:], pd[:])
            S = {nm: prm[:, c0:c1] for nm, (c0, c1) in PCOLS.items()}
            ident = S["ident"]
            maskE25 = S["maskE25"][:G2, :]
            M25 = S["M25"][:T2, :]
            stair2 = S["stair2"][:T2, :]
            stair10 = S["stair10"][:T1, :]
            maskE10 = S["maskE10"][:G1, :]
            M10 = S["M10"][:T1, :]
            sel5 = S["sel5"][:T1, :]
            Mr = S["Mr"][:T1, :]

            h1sb = big.tile([T1, NC1 * FD], BF, tag="h1sb")
            nc.sync.dma_start(h1sb[:], h1d[:])
            h0sb = big.tile([B, FD], BF, tag="h0sb")
            nc.sync.dma_start(h0sb[:], h0d[:])

            # ---- all h2 stream DMAs upfront, 3 queues round-robin ----
            pieces = [None] * NBLK      # k-layout (agg lhsT)
            piecesT = [None] * NBLK     # f-layout (en lhsT)
            QS = (nc.scalar, nc.gpsimd, nc.sync)
            qi = 0
            for b in range(NBLK):
                hbT = h2tp.tile([FD, CPB * T2], BF, tag="h2Tblk", name="hbT")
                QS[qi % 3].dma_start(
                    hbT[:], h2Td[:, b * CPB * T2:(b + 1) * CPB * T2])
                qi += 1
                piecesT[b] = hbT
                hb = h2p.tile([T2, CPB * FD], BF, tag="h2blk", name="hb")
                QS[qi % 3].dma_start(
                    hb[:], h2d[:, b * CPB * FD:(b + 1) * CPB * FD])
                qi += 1
                pieces[b] = hb

            # ---- resident sbuf tiles ----
            x1t = big.tile([128, M1], BF, tag="x1t")
            es1T = big.tile([2, M1], BF, tag="es1T")
            es1m = big.tile([T1, 2 * NQ], BF, tag="es1m")
            es1_arr = big.tile([G2, 2 * NC2], BF, tag="es1_arr")
            h0t = big.tile([128, B], BF, tag="h0t")
            es0m = big.tile([B, 2], BF, tag="es0m")
            es0_arr = big.tile([G1, 2 * NC1], BF, tag="es0_arr")
            aggT1 = big.tile([128, NQ * 160], BF, tag="aggT1")
            aggRow = big.tile([T1, NQ * 2 * FD], BF, tag="aggRow")
            aggT0 = big.tile([128, 2 * G1 * NC1], BF, tag="aggT0")
            esLm = big.tile([B, 2], BF, tag="esLm")
            esL_arr = big.tile([G1, 2 * NC1], BF, tag="esL_arr")
            pre2sb = big.tile([128, 512], BF, tag="pre2sb")
            ysb = big.tile([B, OUT], F32, tag="ysb")

            # long-lived en_L1 psum accumulator (cols 2q+h per flush)
            penL = ppenL.tile([T1, 64], F32, tag="penL")

            # m-ordered per-(q,h) view of aggT1: col = 160q + 2m + h
            def mview():
                return aggT1[:].rearrange(
                    "p (q m h) -> p q h m", q=NQ, h=2)

            # ============ shared softmax (j0 / L1) ============
            def softmax_T1(pen_ap, blk):
                """en+es (already summed) in PSUM [80,32] -> astr [80,256]."""
                e1 = wk.tile([T1, 32], F32, tag=f"e1{blk}")
                nc.vector.tensor_copy(e1[:], pen_ap)
                e2 = wk.tile([T1, 32], F32, tag=f"e2{blk}")
                nc.vector.scalar_tensor_tensor(e2[:], e1[:], NEG, e1[:],
                                               mybir.AluOpType.mult,
                                               mybir.AluOpType.max)
                ex = wk.tile([T1, 32], BF, tag=f"ex{blk}")
                nc.scalar.activation(ex[:], e2[:],
                                     mybir.ActivationFunctionType.Exp)
                denE = ppsm.tile([128, 512], F32, tag="psm", name=f"dE{blk}")
                nc.tensor.matmul(denE[:T1, :32], M10, ex[:])
                rdenE = wk.tile([T1, 32], BF, tag=f"rd{blk}")
                nc.vector.reciprocal(rdenE[:], denE[:T1, :32])
                alpha = wk.tile([T1, 32], BF, tag=f"al{blk}")
                nc.vector.tensor_mul(alpha[:], ex[:], rdenE[:])
                astr = wk.tile([T1, 2 * G1 * NC1], BF, tag=f"as{blk}")
                a4 = alpha[:].rearrange("t (c h) -> t c h", h=2)
                a4 = a4.unsqueeze(2).broadcast_to([T1, NC1, G1, 2])
                s4 = stair10.rearrange("t (g h) -> t g h", h=2).unsqueeze(1)
                s4 = s4.broadcast_to([T1, NC1, G1, 2])
                nc.vector.tensor_mul(
                    astr[:].rearrange("t (c g h) -> t c g h", g=G1, h=2),
                    a4, s4)
                return astr

            # ================= j0 prologue (h1/h0 only; runs under the
            # initial DMA wave) =================
            for grp in range(4):
                pt = ppt.tile([128, 1024], BF, tag="pt", name="pt_x1t")
                for j in range(4):
                    cc = 4 * grp + j
                    nc.tensor.transpose(pt[:, 128 * j:128 * j + 80],
                                        h1sb[:, cc * FD:(cc + 1) * FD],
                                        ident[:T1, :T1])
                nc.vector.tensor_copy(
                    x1t[:, 320 * grp:320 * (grp + 1)].rearrange(
                        "p (j c) -> p j c", c=80),
                    pt[:, :512].rearrange("p (j c) -> p j c", c=128)[:, :, :80])
            for s in range(4):
                ps = ppsm.tile([128, 512], F32, tag="psm", name="ps_es1T")
                nc.tensor.matmul(ps[:2, :320], S["u0"][:],
                                 x1t[:, 320 * s:320 * (s + 1)])
                nc.vector.tensor_copy(es1T[:, 320 * s:320 * (s + 1)],
                                      ps[:2, :320])
            pm = ppt.tile([128, 1024], BF, tag="pt", name="pt_es1m")
            for q in range(NQ):
                nc.tensor.transpose(pm[:T1, 2 * q:2 * q + 2],
                                    es1T[:, 80 * q:80 * (q + 1)],
                                    ident[:2, :2])
            nc.vector.tensor_copy(es1m[:], pm[:T1, :2 * NQ])
            R1 = wk.tile([T1, 512], BF, tag="R1")
            a3 = es1m[:].rearrange("t (q h) -> t q h", h=2)
            a4 = a3.unsqueeze(3).broadcast_to([T1, NQ, 2, 16])
            m4 = Mr.unsqueeze(1).unsqueeze(1).broadcast_to([T1, NQ, 2, 16])
            nc.vector.tensor_mul(
                R1[:].rearrange("t (q h r) -> t q h r", h=2, r=16), a4, m4)
            psE = ppsm.tile([128, 512], F32, tag="psm", name="psE")
            for q in range(NQ):
                nc.tensor.matmul(psE[:G2, 32 * q:32 * (q + 1)], sel5,
                                 R1[:, 32 * q:32 * (q + 1)])
            nc.vector.tensor_copy(
                es1_arr[:].rearrange("g (h q r) -> g q h r", q=NQ, r=16),
                psE[:G2, :].rearrange("g (q h r) -> g q h r", h=2, r=16))

            pt0 = ppt.tile([128, 1024], BF, tag="pt", name="pt_h0t")
            nc.tensor.transpose(pt0[:, :128], h0sb[:], ident[:])
            nc.vector.tensor_copy(h0t[:], pt0[:, :128])
            ps0 = ppsm.tile([128, 512], F32, tag="psm", name="ps_es0")
            nc.tensor.matmul(ps0[:2, :128], S["u0"][:], h0t[:])
            es0Tsb = wk.tile([2, B], BF, tag="es0Tsb")
            nc.vector.tensor_copy(es0Tsb[:], ps0[:2, :128])
            pt1 = ppt.tile([128, 1024], BF, tag="pt", name="pt_es0m")
            nc.tensor.transpose(pt1[:B, :2], es0Tsb[:], ident[:2, :2])
            nc.vector.tensor_copy(es0m[:], pt1[:B, :2])
            R0 = wk.tile([B, 2 * NC1], BF, tag="R0")
            b3 = es0m[:].unsqueeze(1).broadcast_to([B, NC1, 2])
            c3 = S["maskC16"].unsqueeze(2).broadcast_to([B, NC1, 2])
            nc.vector.tensor_mul(
                R0[:].rearrange("m (c h) -> m c h", h=2), b3, c3)
            psA = ppsm.tile([128, 512], F32, tag="psm", name="psA")
            nc.tensor.matmul(psA[:G1, :32], S["sel8"], R0[:])
            nc.vector.tensor_copy(es0_arr[:], psA[:G1, :32])

            # j0 GAT: en1 into psum, softmax, aggregation, esL machinery
            pe1 = ppen.tile([T2, 64], F32, tag="pen", name="pe1")
            nc.tensor.matmul(pe1[:T1, :2 * NC1], maskE10, es0_arr[:],
                             start=True, stop=False, skip_group_check=True)
            for cc in range(NC1):
                nc.tensor.matmul(pe1[:T1, 2 * cc:2 * cc + 2],
                                 x1t[:, 80 * cc:80 * (cc + 1)], S["v0"][:],
                                 start=False, stop=True,
                                 skip_group_check=True)
            astr0 = softmax_T1(pe1[:T1, :2 * NC1], "j0")
            pg0 = ppagg.tile([128, 256], F32, tag="pagg", name="pagg0")
            for cc in range(NC1):
                nc.tensor.matmul(pg0[:, 16 * cc:16 * cc + 16],
                                 h1sb[:, cc * FD:(cc + 1) * FD],
                                 astr0[:, 16 * cc:16 * cc + 16])
            nc.scalar.copy(aggT0[:], pg0[:, :256])
            psL = ppsm.tile([128, 512], F32, tag="psm", name="psL")
            t0v = aggT0[:].rearrange("p (m h) -> p h m", h=2)
            sls = [t0v[:, h, :] for h in range(H)]
            for h in range(H):
                nc.tensor.matmul(psL[:2, :B], S["Wu1"][:, 2 * h:2 * h + 2],
                                 sls[h], start=(h == 0), stop=(h == 1))
            esLTsb = wk.tile([2, B], BF, tag="esLTsb")
            nc.scalar.copy(esLTsb[:], psL[:2, :B])
            ptL = ppt.tile([128, 1024], BF, tag="pt", name="pt_esLm")
            nc.tensor.transpose(ptL[:B, :2], esLTsb[:], ident[:2, :2])
            nc.scalar.copy(esLm[:], ptL[:B, :2])
            RL = wk.tile([B, 2 * NC1], BF, tag="RL")
            b3 = esLm[:].unsqueeze(1).broadcast_to([B, NC1, 2])
            c3 = S["maskC16"].unsqueeze(2).broadcast_to([B, NC1, 2])
            nc.vector.tensor_mul(
                RL[:].rearrange("m (c h) -> m c h", h=2), b3, c3)
            psB = ppsm.tile([128, 512], F32, tag="psm", name="psB")
            nc.tensor.matmul(psB[:G1, :32], S["sel8"], RL[:])
            nc.scalar.copy(esL_arr[:], psB[:G1, :32])
            # open the en_L1 accumulator with the es_L1 expansion
            nc.tensor.matmul(penL[:, :32], maskE10, esL_arr[:],
                             start=True, stop=False, skip_group_check=True)

            # ============ j1 per-block stages ============
            pens = [None] * NBLK
            exs = [None] * NBLK
            astrs = [None] * NBLK

            def stage1_en(b):
                hbT = piecesT[b]
                pen = ppen.tile([T2, 64], F32, tag="pen", name="pen1")
                es_v = es1_arr[:].rearrange("g (h c) -> g c h", h=2)[
                    :, CPB * b:CPB * (b + 1), :]
                nc.tensor.matmul(pen[:, :64], maskE25, es_v,
                                 start=True, stop=False,
                                 skip_group_check=True)
                for cl in range(CPB):
                    nc.tensor.matmul(pen[:, 2 * cl:2 * cl + 2],
                                     hbT[:, T2 * cl:T2 * (cl + 1)],
                                     S["v0"][:], start=False, stop=True,
                                     skip_group_check=True)
                pens[b] = pen

            def stage1_sm1(b):
                pen = pens[b]
                e1 = wk.tile([T2, 64], F32, tag="e1j1")
                nc.vector.tensor_copy(e1[:], pen[:, :64])
                e2 = wk.tile([T2, 64], F32, tag="e2j1")
                nc.vector.scalar_tensor_tensor(e2[:], e1[:], NEG, e1[:],
                                               mybir.AluOpType.mult,
                                               mybir.AluOpType.max)
                ex = wk.tile([T2, 64], BF, tag="exj1")
                nc.scalar.activation(ex[:], e2[:],
                                     mybir.ActivationFunctionType.Exp)
                exs[b] = ex

            def sm2a(b):
                denE = ppsm.tile([128, 512], F32, tag="psm", name="dE1")
                nc.tensor.matmul(denE[:T2, :64], M25, exs[b][:])
                rdenE = wk.tile([T2, 64], BF, tag="rden1")
                nc.vector.reciprocal(rdenE[:], denE[:T2, :64])
                alpha = wk.tile([T2, 64], BF, tag="al1")
                nc.vector.tensor_mul(alpha[:], exs[b][:], rdenE[:])
                astr = wk.tile([T2, CPB * 10], BF, tag="as1")
                av = alpha[:].rearrange("t (hf r h) -> t hf r h", hf=2, h=2)
                s4 = stair2.rearrange("t (g h) -> t g h", h=2)
                s4 = s4.unsqueeze(2).broadcast_to([T2, G2, 16, 2])
                ov = astr[:].rearrange("t (hf g r h) -> t hf g r h",
                                       g=G2, r=16, h=2)
                for hf in range(2):
                    a4 = av[:, hf, :, :].unsqueeze(1).broadcast_to(
                        [T2, G2, 16, 2])
                    nc.vector.tensor_mul(ov[:, hf, :, :, :], a4, s4)
                astrs[b] = astr

            def sm2c(b):
                hb = pieces[b]
                astr = astrs[b]
                for half in range(2):
                    q = 2 * b + half
                    pagg = ppagg.tile([128, 256], F32, tag="pagg",
                                      name="pagg1")
                    for rr in range(16):
                        cl = 16 * half + rr
                        for g in range(G2):
                            c0 = 160 * half + 32 * g + 2 * rr
                            nc.tensor.matmul(pagg[:, 32 * g + 2 * rr:
                                                  32 * g + 2 * rr + 2],
                                             hb[:, cl * FD:(cl + 1) * FD],
                                             astr[:, c0:c0 + 2])
                    nc.scalar.copy(aggT1[:, 160 * q:160 * (q + 1)],
                                   pagg[:, :160])

            def stage_fl(b):
                """aggRow transposes + en_L1 for block b's two flushes."""
                mv = mview()
                for half in range(2):
                    q = 2 * b + half
                    pagr = ppt.tile([128, 1024], BF, tag="pt", name="pagr")
                    sls = [mv[:, q, h, :] for h in range(H)]
                    for h in range(H):
                        nc.tensor.transpose(pagr[:T1, 128 * h:128 * (h + 1)],
                                            sls[h], ident[:])
                    for h in range(H):
                        nc.tensor.matmul(penL[:, 2 * q:2 * q + 2], sls[h],
                                         S["Wv1"][:, 2 * h:2 * h + 2],
                                         start=False, stop=(h == 1),
                                         skip_group_check=True)
                    if half:
                        nc.scalar.copy(aggRow[:, 256 * q:256 * (q + 1)],
                                       pagr[:T1, :256])
                    else:
                        nc.vector.tensor_copy(
                            aggRow[:, 256 * q:256 * (q + 1)],
                            pagr[:T1, :256])

            # ---- emission ----
            for it in range(NBLK + 2):
                cur = it if it < NBLK else None
                prv = it - 1 if 1 <= it <= NBLK else None
                if cur is not None:
                    stage1_en(cur)
                if prv is not None:
                    sm2a(prv)
                if cur is not None:
                    stage1_sm1(cur)
                if prv is not None:
                    sm2c(prv)
                if it >= 2:
                    stage_fl(it - 2)

            # ============ L1 tail ============
            astrL = softmax_T1(penL[:T1, :32], "L1")
            pr2 = []
            for h in range(H):
                p = ppagg.tile([128, 256], F32, tag="pagg", name=f"pre2_{h}")
                pr2.append(p)
            for cc in range(NC1):
                for h in range(H):
                    nc.tensor.matmul(
                        pr2[h][:, 16 * cc:16 * cc + 16],
                        aggRow[:, 256 * cc + 128 * h:256 * cc + 128 * (h + 1)],
                        astrL[:, 16 * cc:16 * cc + 16])
            nc.vector.tensor_copy(pre2sb[:, :256], pr2[0][:, :256])
            nc.scalar.copy(pre2sb[:, 256:], pr2[1][:, :256])
            py = ppt.tile([128, 128], F32, tag="pt", name="py")
            p2v = pre2sb[:].rearrange("p (h m hp) -> p h hp m",
                                      h=2, hp=2)
            idx = 0
            for hp in range(H):
                for h in range(H):
                    nc.tensor.matmul(py[:, :128], p2v[:, h, hp, :],
                                     S["H4"][:, (2 * hp + h) * 128:
                                             (2 * hp + h + 1) * 128],
                                     start=(idx == 0), stop=(idx == 3))
                    idx += 1
            nc.vector.tensor_copy(ysb[:], py[:])
            nc.sync.dma_start(yd[:], ysb[:])

    if split_waits:
        _split_multi_waits(nc)
    return nc


_PROG = None


def kernel(**inputs):
    global _PROG
    _install_drain_patch()
    prm = host_params(inputs["W0"], inputs["a_self0"], inputs["a_neigh0"],
                      inputs["W1"], inputs["a_self1"], inputs["a_neigh1"],
                      inputs["fc_W"])
    h0pk, h1pk, h2pk, h2T = pack_inputs(inputs["h0"], inputs["h1"],
                                        inputs["h2"])
    if _PROG is None:
        _PROG = build_program()
    nc = _PROG
    in_maps = []
    for c in range(NCORES):
        in_maps.append({"h0pk": h0pk[c], "h1pk": h1pk[c], "h2pk": h2pk[c],
                        "h2T": h2T[c], "prm": prm})
    core_ids = list(range(NCORES))
    last = None
    for _attempt in range(3):
        try:
            res = run_bass_kernel_spmd(nc, in_maps, core_ids)
            out = np.concatenate([np.asarray(res.results[c]["y"])
                                  for c in core_ids], axis=0)
            return out.astype(np.float32)
        except Exception as e:   # transient device-unrecoverable happens
            last = e
    raise last


# revision 4
# speedup vs baseline: 2.7379x; 1.1429x over previous
"""Trainium2 Bass kernel for hierarchical 2-layer GAT (nn_GAT_20383914787079).

Data-parallel over 8 NeuronCores: each core owns B/8 = 128 root nodes and
their full neighbor subtree (1280 level-1 rows, 32000 level-2 rows).

Design notes:
  * All hierarchy inputs are packed to bf16 tile layouts on the host. The
    level-2 features are uploaded in BOTH layouts the kernel needs --
    k-on-partitions (for the alpha-weighted aggregation matmuls) and
    f-on-partitions (for the attention-logit matmuls) -- which removes every
    on-chip transpose/copy of the big stream. Total HBM bytes equal the
    original fp32 single-layout upload.
  * DMA is issued round-robin across the three DGE queues (SP / Activation
    HWDGE and the GPSIMD SWDGE path) so transfers overlap.
  * Every projection is folded into host-precomputed parameters:
      en_L1 = aggT1 @ (W0^T v1),  es_L1 = aggT0 @ (W0^T u1),
      y     = sum_{h',h} pre2_{h',h} @ H[h',h],  H = W0*W1*fcW folded,
    so the per-head GAT projections (g0/g1) are never materialized.
  * Softmax denominators are expanded with a single fused mask matmul
    (denE = M @ ex, M[t,p] = same-group indicator), halving the
    cross-engine hops in the per-block dependency chain.
  * Attention-logit rearranges (es -> grouped arrays) are done with tiny
    PE matmuls against 0/1 selector masks instead of descriptor-heavy DMAs.

j1 chunk remap: chunk ci (125 rows = 5 groups of 25 neighbors) owns level-1
rows m = 80*(ci//16) + 16*g + (ci%16); flush q = 16 chunks covers the
contiguous m-window [80q, 80q+80).

aggT1 column layout: col = 160q + 10r + 2g + h; the m-ordered per-head view
is [g stride 2][r stride 10], giving m = 80q + 16g + r.
"""
import numpy as np
import ml_dtypes

import concourse.bass as bass
import concourse.tile as tile
from concourse import mybir
from concourse.vector_clock import ScopedClock
from concourse.bass_utils import run_bass_kernel_spmd

BF = mybir.dt.bfloat16
F32 = mybir.dt.float32
NPBF = ml_dtypes.bfloat16

NEG = 0.2
NCORES = 8
B, FD, D, H, OUT = 128, 128, 128, 2, 128   # per-core roots, dims
R0, R1 = 10, 25
M1 = B * R0                                 # 1280
M2 = M1 * R1                                # 32000

T2, NC2, G2 = 125, 256, 5                   # j1 chunks
T1, NC1, G1 = 80, 16, 8                     # j0/L1 chunks
NQ = 16                                     # j1 flushes (80-m windows)
NBLK = 8                                    # h2 stream blocks (32 chunks each)
CPB = NC2 // NBLK                           # 32 chunks per block


def _install_drain_patch():
    """This container's walrus rejects >1 sync-wait per instruction; split the
    Tile tail-drain waits across SP NoOps."""
    def _patched(self, tick_clock, wait_clock):
        nc = self.nc
        probe = nc.sync.nop(nofuse=True, hint="drain_wait_split")
        wait_clock.add_sem_waits(probe.ins,
                                 ScopedClock({None: tick_clock.global_clock}))
        si = probe.ins.sync_info
        waits = list(si.on_wait) if si is not None and si.on_wait else []
        if len(waits) > 1:
            si.on_wait = [waits[0]]
            for wi in waits[1:]:
                n2 = nc.sync.nop(nofuse=True, hint="drain_wait_split")
                if n2.ins.sync_info is None:
                    n2.ins.sync_info = mybir.SyncInfo(on_wait=[wi], on_update=[])
                else:
                    n2.ins.sync_info.on_wait = [wi]
        nc.sync.drain()
        nc.all_engine_barrier()
        popped = nc._tile_sem_poison_stack.pop()
        assert popped is self._sem_poison
        nc.clear_and_free_semaphores(list(self.sems.allocated().values()))
        nc.all_engine_barrier()

    tile.TileContext._drain_and_barrier = _patched


def _split_multi_waits(nc):
    """Walrus here allows only one sync-wait per instruction: hoist extra
    waits onto same-engine NoOps inserted immediately before."""
    nid = [0]
    for fn in nc.m.functions:
        for bb in fn.blocks:
            insts = bb.instructions
            i = 0
            while i < len(insts):
                inst = insts[i]
                si = inst.sync_info
                if si is not None and si.on_wait and len(si.on_wait) > 1:
                    waits = list(si.on_wait)
                    si.on_wait = [waits[-1]]
                    for w in waits[:-1]:
                        nid[0] += 1
                        nop = mybir.InstNoOp(
                            name=f"waitsplit-{nid[0]}", ins=[], outs=[],
                            sync_info=mybir.SyncInfo(on_wait=[w], on_update=[]))
                        nop.engine = inst.engine
                        insts.insert(i, nop)
                        i += 1
                i += 1


# ---------------- host-side parameter folding + input packing --------------

def _cols():
    off = [0]
    d = {}

    def put(nm, n):
        d[nm] = (off[0], off[0] + n)
        off[0] += n
    put("u0", 2); put("v0", 2); put("Wu1", 4); put("Wv1", 4)
    put("H4", 512)
    put("maskE25", T2)       # [5,125]  expand es to rows
    put("M25", T2)           # [125,125] same-group indicator (denE)
    put("stair2", 2 * G2)    # [125,10] (t//25==g), (g,h) pairs
    put("stair10", 2 * G1)   # [80,16]  (t//10==g), (g,h) pairs
    put("maskE10", T1)       # [8,80]
    put("M10", T1)           # [80,80]
    put("ident", 128)
    put("sel5", G2)          # [80,5]  (t//16==g)
    put("Mr", 16)            # [80,16] (t%16==r)
    put("sel8", G1)          # [128,8] (m0%8==g)
    put("maskC16", NC1)      # [128,16] (m0//8==c)
    return d, off[0]


PCOLS, PN = _cols()


def host_params(W0, a_s0, a_n0, W1, a_s1, a_n1, fc_W):
    W0 = np.float64(W0); W1 = np.float64(W1)
    u0 = np.einsum("hfd,hd->fh", W0, np.float64(a_s0))
    v0 = np.einsum("hfd,hd->fh", W0, np.float64(a_n0))
    u1 = np.einsum("hcd,hd->ch", W1, np.float64(a_s1))
    v1 = np.einsum("hcd,hd->ch", W1, np.float64(a_n1))
    Wu1 = np.zeros((128, 4)); Wv1 = np.zeros((128, 4))
    for h in range(H):
        Wu1[:, 2 * h:2 * h + 2] = W0[h] @ u1[128 * h:128 * (h + 1), :]
        Wv1[:, 2 * h:2 * h + 2] = W0[h] @ v1[128 * h:128 * (h + 1), :]
    fcW = np.float64(fc_W)
    H4 = np.zeros((128, 512))          # col = (2h'+h)*128 + o
    for hp in range(H):
        Gp = W1[hp] @ fcW[128 * hp:128 * (hp + 1), :]
        for h in range(H):
            H4[:, (2 * hp + h) * 128:(2 * hp + h + 1) * 128] = \
                W0[h] @ Gp[128 * h:128 * (h + 1), :]

    blob = np.zeros((128, PN))

    def put(nm, a):
        c0, c1 = PCOLS[nm]
        blob[:a.shape[0], c0:c1] = a
    put("u0", u0); put("v0", v0); put("Wu1", Wu1); put("Wv1", Wv1)
    put("H4", H4)
    t2 = np.arange(T2); t1 = np.arange(T1); m0 = np.arange(128)
    m25 = (t2[:, None] // R1 == np.arange(G2)[None, :]).astype(float)
    put("maskE25", m25.T)
    put("M25", (t2[:, None] // R1 == t2[None, :] // R1).astype(float))
    put("stair2", np.repeat(m25, 2, axis=1))
    m10 = (t1[:, None] // R0 == np.arange(G1)[None, :]).astype(float)
    put("stair10", np.repeat(m10, 2, axis=1))
    put("maskE10", m10.T)
    put("M10", (t1[:, None] // R0 == t1[None, :] // R0).astype(float))
    put("ident", np.eye(128))
    put("sel5", (t1[:, None] // 16 == np.arange(G2)[None, :]).astype(float))
    put("Mr", (t1[:, None] % 16 == np.arange(16)[None, :]).astype(float))
    put("sel8", (m0[:, None] % 8 == np.arange(G1)[None, :]).astype(float))
    put("maskC16", (m0[:, None] // 8 == np.arange(NC1)[None, :]).astype(float))
    return np.ascontiguousarray(blob.astype(NPBF))


_CI = np.arange(NC2)
_MMAP = 80 * (_CI[:, None] // 16) + 16 * np.arange(G2)[None, :] + (_CI[:, None] % 16)


def pack_inputs(h0, h1, h2):
    """Full inputs -> per-core packed bf16 arrays (both h2 layouts)."""
    h0 = np.asarray(h0, np.float32); h1 = np.asarray(h1, np.float32)
    h2 = np.asarray(h2, np.float32)
    a2 = h2.reshape(NCORES, M1, R1, FD)
    pk = a2[:, _MMAP]                          # [8, 256, 5, 25, 128]
    pk = pk.transpose(0, 2, 3, 1, 4)           # [8, 5, 25, 256, 128]
    h2pk = np.ascontiguousarray(
        pk.reshape(NCORES, T2, NC2 * FD)).astype(NPBF)
    # f-on-partitions layout: col = 125*ci + p
    h2T = np.ascontiguousarray(
        pk.reshape(NCORES, T2, NC2, FD).transpose(0, 3, 2, 1)
        .reshape(NCORES, FD, NC2 * T2)).astype(NPBF)
    h1pk = np.ascontiguousarray(
        h1.reshape(NCORES, NC1, T1, FD).transpose(0, 2, 1, 3)
        .reshape(NCORES, T1, NC1 * FD)).astype(NPBF)
    h0pk = np.ascontiguousarray(h0.reshape(NCORES, B, FD)).astype(NPBF)
    return h0pk, h1pk, h2pk, h2T


# ----------------------------- device program ------------------------------

def build_program(split_waits=True):
    nc = bass.Bass()
    dp = nc.declare_dram_parameter
    h0d = dp("h0pk", [B, FD], BF, isOutput=False)
    h1d = dp("h1pk", [T1, NC1 * FD], BF, isOutput=False)
    h2d = dp("h2pk", [T2, NC2 * FD], BF, isOutput=False)
    h2Td = dp("h2T", [FD, NC2 * T2], BF, isOutput=False)
    pd = dp("prm", [128, PN], BF, isOutput=False)
    yd = dp("y", [B, OUT], F32, isOutput=True)

    with tile.TileContext(nc) as tc, \
         nc.allow_low_precision(reason="bf16 data path; 2e-2 tolerance"):
        with (tc.tile_pool(name="big", bufs=1) as big,
              tc.tile_pool(name="h2p", bufs=NBLK) as h2p,
              tc.tile_pool(name="h2tp", bufs=NBLK) as h2tp,
              tc.tile_pool(name="wk", bufs=6) as wk,
              tc.tile_pool(name="ppt", bufs=2, space="PSUM") as ppt,
              tc.tile_pool(name="ppen", bufs=2, space="PSUM") as ppen,
              tc.tile_pool(name="ppsm", bufs=1, space="PSUM") as ppsm,
              tc.tile_pool(name="ppagg", bufs=2, space="PSUM") as ppagg,
              tc.tile_pool(name="ppenL", bufs=1, space="PSUM") as ppenL):

            # ---- param / small-input DMAs (SP queue) ----
            prm = big.tile([128, PN], BF, tag="prm")
            nc.sync.dma_start(prm[:], pd[:])
            S = {nm: prm[:, c0:c1] for nm, (c0, c1) in PCOLS.items()}
            ident = S["ident"]
            maskE25 = S["maskE25"][:G2, :]
            M25 = S["M25"][:T2, :]
            stair2 = S["stair2"][:T2, :]
            stair10 = S["stair10"][:T1, :]
            maskE10 = S["maskE10"][:G1, :]
            M10 = S["M10"][:T1, :]
            sel5 = S["sel5"][:T1, :]
            Mr = S["Mr"][:T1, :]

            h1sb = big.tile([T1, NC1 * FD], BF, tag="h1sb")
            nc.sync.dma_start(h1sb[:], h1d[:])
            h0sb = big.tile([B, FD], BF, tag="h0sb")
            nc.sync.dma_start(h0sb[:], h0d[:])

            # ---- all h2 stream DMAs upfront, 3 queues round-robin ----
            pieces = [None] * NBLK      # k-layout (agg lhsT)
            piecesT = [None] * NBLK     # f-layout (en lhsT)
            QS = (nc.scalar, nc.gpsimd, nc.sync)
            qi = 0
            for b in range(NBLK):
                hbT = h2tp.tile([FD, CPB * T2], BF, tag="h2Tblk", name="hbT")
                QS[qi % 3].dma_start(
                    hbT[:], h2Td[:, b * CPB * T2:(b + 1) * CPB * T2])
                qi += 1
                piecesT[b] = hbT
            for b in range(NBLK):
                hb = h2p.tile([T2, CPB * FD], BF, tag="h2blk", name="hb")
                q = nc.gpsimd if b == NBLK - 1 else QS[qi % 3]
                q.dma_start(
                    hb[:], h2d[:, b * CPB * FD:(b + 1) * CPB * FD])
                qi += 1
                pieces[b] = hb

            # ---- resident sbuf tiles ----
            x1t = big.tile([128, M1], BF, tag="x1t")
            es1T = big.tile([2, M1], BF, tag="es1T")
            es1m = big.tile([T1, 2 * NQ], BF, tag="es1m")
            es1_arr = big.tile([G2, 2 * NC2], BF, tag="es1_arr")
            h0t = big.tile([128, B], BF, tag="h0t")
            es0m = big.tile([B, 2], BF, tag="es0m")
            es0_arr = big.tile([G1, 2 * NC1], BF, tag="es0_arr")
            aggT1 = big.tile([128, NQ * 160], BF, tag="aggT1")
            aggRow = big.tile([T1, NQ * 2 * FD], BF, tag="aggRow")
            aggT0 = big.tile([128, 2 * G1 * NC1], BF, tag="aggT0")
            esLm = big.tile([B, 2], BF, tag="esLm")
            esL_arr = big.tile([G1, 2 * NC1], BF, tag="esL_arr")
            pre2sb = big.tile([128, 512], BF, tag="pre2sb")
            ysb = big.tile([B, OUT], F32, tag="ysb")

            # long-lived en_L1 psum accumulator (cols 2q+h per flush)
            penL = ppenL.tile([T1, 64], F32, tag="penL")

            # m-ordered per-(q,h) view of aggT1: col = 160q + 2m + h
            def mview():
                return aggT1[:].rearrange(
                    "p (q m h) -> p q h m", q=NQ, h=2)

            # ============ shared softmax (j0 / L1) ============
            def softmax_T1(pen_ap, blk):
                """en+es (already summed) in PSUM [80,32] -> astr [80,256]."""
                e1 = wk.tile([T1, 32], F32, tag=f"e1{blk}")
                nc.vector.tensor_copy(e1[:], pen_ap)
                e2 = wk.tile([T1, 32], F32, tag=f"e2{blk}")
                nc.vector.scalar_tensor_tensor(e2[:], e1[:], NEG, e1[:],
                                               mybir.AluOpType.mult,
                                               mybir.AluOpType.max)
                ex = wk.tile([T1, 32], BF, tag=f"ex{blk}")
                nc.scalar.activation(ex[:], e2[:],
                                     mybir.ActivationFunctionType.Exp)
                denE = ppsm.tile([128, 512], F32, tag="psm", name=f"dE{blk}")
                nc.tensor.matmul(denE[:T1, :32], M10, ex[:])
                rdenE = wk.tile([T1, 32], BF, tag=f"rd{blk}")
                nc.vector.reciprocal(rdenE[:], denE[:T1, :32])
                alpha = wk.tile([T1, 32], BF, tag=f"al{blk}")
                nc.vector.tensor_mul(alpha[:], ex[:], rdenE[:])
                astr = wk.tile([T1, 2 * G1 * NC1], BF, tag=f"as{blk}")
                a4 = alpha[:].rearrange("t (c h) -> t c h", h=2)
                a4 = a4.unsqueeze(2).broadcast_to([T1, NC1, G1, 2])
                s4 = stair10.rearrange("t (g h) -> t g h", h=2).unsqueeze(1)
                s4 = s4.broadcast_to([T1, NC1, G1, 2])
                nc.vector.tensor_mul(
                    astr[:].rearrange("t (c g h) -> t c g h", g=G1, h=2),
                    a4, s4)
                return astr

            # ================= j0 prologue (h1/h0 only; runs under the
            # initial DMA wave) =================
            for grp in range(4):
                pt = ppt.tile([128, 1024], BF, tag="pt", name="pt_x1t")
                for j in range(4):
                    cc = 4 * grp + j
                    nc.tensor.transpose(pt[:, 128 * j:128 * j + 80],
                                        h1sb[:, cc * FD:(cc + 1) * FD],
                                        ident[:T1, :T1])
                nc.vector.tensor_copy(
                    x1t[:, 320 * grp:320 * (grp + 1)].rearrange(
                        "p (j c) -> p j c", c=80),
                    pt[:, :512].rearrange("p (j c) -> p j c", c=128)[:, :, :80])
            for s in range(4):
                ps = ppagg.tile([128, 512], F32, tag="pagg", name="ps_es1T")
                nc.tensor.matmul(ps[:2, :320], S["u0"][:],
                                 x1t[:, 320 * s:320 * (s + 1)])
                nc.vector.tensor_copy(es1T[:, 320 * s:320 * (s + 1)],
                                      ps[:2, :320])
            pm = ppt.tile([128, 1024], BF, tag="pt", name="pt_es1m")
            for q in range(NQ):
                nc.tensor.transpose(pm[:T1, 2 * q:2 * q + 2],
                                    es1T[:, 80 * q:80 * (q + 1)],
                                    ident[:2, :2])
            nc.vector.tensor_copy(es1m[:], pm[:T1, :2 * NQ])
            R1 = wk.tile([T1, 512], BF, tag="R1")
            a3 = es1m[:].rearrange("t (q h) -> t q h", h=2)
            a4 = a3.unsqueeze(3).broadcast_to([T1, NQ, 2, 16])
            m4 = Mr.unsqueeze(1).unsqueeze(1).broadcast_to([T1, NQ, 2, 16])
            nc.vector.tensor_mul(
                R1[:].rearrange("t (q h r) -> t q h r", h=2, r=16), a4, m4)
            psE = ppagg.tile([128, 512], F32, tag="pagg", name="psE")
            for q in range(NQ):
                nc.tensor.matmul(psE[:G2, 32 * q:32 * (q + 1)], sel5,
                                 R1[:, 32 * q:32 * (q + 1)])
            nc.vector.tensor_copy(
                es1_arr[:].rearrange("g (h q r) -> g q h r", q=NQ, r=16),
                psE[:G2, :].rearrange("g (q h r) -> g q h r", h=2, r=16))

            pt0 = ppt.tile([128, 1024], BF, tag="pt", name="pt_h0t")
            nc.tensor.transpose(pt0[:, :128], h0sb[:], ident[:])
            nc.vector.tensor_copy(h0t[:], pt0[:, :128])
            ps0 = ppagg.tile([128, 512], F32, tag="pagg", name="ps_es0")
            nc.tensor.matmul(ps0[:2, :128], S["u0"][:], h0t[:])
            es0Tsb = wk.tile([2, B], BF, tag="es0Tsb")
            nc.vector.tensor_copy(es0Tsb[:], ps0[:2, :128])
            pt1 = ppt.tile([128, 1024], BF, tag="pt", name="pt_es0m")
            nc.tensor.transpose(pt1[:B, :2], es0Tsb[:], ident[:2, :2])
            nc.vector.tensor_copy(es0m[:], pt1[:B, :2])
            R0 = wk.tile([B, 2 * NC1], BF, tag="R0")
            b3 = es0m[:].unsqueeze(1).broadcast_to([B, NC1, 2])
            c3 = S["maskC16"].unsqueeze(2).broadcast_to([B, NC1, 2])
            nc.vector.tensor_mul(
                R0[:].rearrange("m (c h) -> m c h", h=2), b3, c3)
            psA = ppagg.tile([128, 512], F32, tag="pagg", name="psA")
            nc.tensor.matmul(psA[:G1, :32], S["sel8"], R0[:])
            nc.vector.tensor_copy(es0_arr[:], psA[:G1, :32])

            # j0 GAT: en1 into psum, softmax, aggregation, esL machinery
            pe1 = ppen.tile([T2, 64], F32, tag="pen", name="pe1")
            nc.tensor.matmul(pe1[:T1, :2 * NC1], maskE10, es0_arr[:],
                             start=True, stop=False, skip_group_check=True)
            for cc in range(NC1):
                nc.tensor.matmul(pe1[:T1, 2 * cc:2 * cc + 2],
                                 x1t[:, 80 * cc:80 * (cc + 1)], S["v0"][:],
                                 start=False, stop=True,
                                 skip_group_check=True)
            astr0 = softmax_T1(pe1[:T1, :2 * NC1], "j0")
            pg0 = ppagg.tile([128, 256], F32, tag="pagg", name="pagg0")
            for cc in range(NC1):
                nc.tensor.matmul(pg0[:, 16 * cc:16 * cc + 16],
                                 h1sb[:, cc * FD:(cc + 1) * FD],
                                 astr0[:, 16 * cc:16 * cc + 16])
            nc.scalar.copy(aggT0[:], pg0[:, :256])
            psL = ppagg.tile([128, 512], F32, tag="pagg", name="psL")
            t0v = aggT0[:].rearrange("p (m h) -> p h m", h=2)
            sls = [t0v[:, h, :] for h in range(H)]
            for h in range(H):
                nc.tensor.matmul(psL[:2, :B], S["Wu1"][:, 2 * h:2 * h + 2],
                                 sls[h], start=(h == 0), stop=(h == 1))
            esLTsb = wk.tile([2, B], BF, tag="esLTsb")
            nc.scalar.copy(esLTsb[:], psL[:2, :B])
            ptL = ppt.tile([128, 1024], BF, tag="pt", name="pt_esLm")
            nc.tensor.transpose(ptL[:B, :2], esLTsb[:], ident[:2, :2])
            nc.scalar.copy(esLm[:], ptL[:B, :2])
            RL = wk.tile([B, 2 * NC1], BF, tag="RL")
            b3 = esLm[:].unsqueeze(1).broadcast_to([B, NC1, 2])
            c3 = S["maskC16"].unsqueeze(2).broadcast_to([B, NC1, 2])
            nc.vector.tensor_mul(
                RL[:].rearrange("m (c h) -> m c h", h=2), b3, c3)
            psB = ppagg.tile([128, 512], F32, tag="pagg", name="psB")
            nc.tensor.matmul(psB[:G1, :32], S["sel8"], RL[:])
            nc.scalar.copy(esL_arr[:], psB[:G1, :32])
            # open the en_L1 accumulator with the es_L1 expansion
            nc.tensor.matmul(penL[:, :32], maskE10, esL_arr[:],
                             start=True, stop=False, skip_group_check=True)

            # ============ j1 per-block stages ============
            pens = [None] * NBLK
            exs = [None] * NBLK
            astrs = [None] * NBLK

            def stage1_en(b):
                hbT = piecesT[b]
                pen = ppen.tile([T2, 64], F32, tag="pen", name="pen1")
                es_v = es1_arr[:].rearrange("g (h c) -> g c h", h=2)[
                    :, CPB * b:CPB * (b + 1), :]
                nc.tensor.matmul(pen[:, :64], maskE25, es_v,
                                 start=True, stop=False,
                                 skip_group_check=True)
                for cl in range(CPB):
                    nc.tensor.matmul(pen[:, 2 * cl:2 * cl + 2],
                                     hbT[:, T2 * cl:T2 * (cl + 1)],
                                     S["v0"][:], start=False, stop=True,
                                     skip_group_check=True)
                pens[b] = pen

            def stage1_sm1(b):
                pen = pens[b]
                e1 = wk.tile([T2, 64], F32, tag="e1j1")
                nc.vector.tensor_copy(e1[:], pen[:, :64])
                e2 = wk.tile([T2, 64], F32, tag="e2j1")
                nc.vector.scalar_tensor_tensor(e2[:], e1[:], NEG, e1[:],
                                               mybir.AluOpType.mult,
                                               mybir.AluOpType.max)
                ex = wk.tile([T2, 64], BF, tag="exj1")
                nc.scalar.activation(ex[:], e2[:],
                                     mybir.ActivationFunctionType.Exp)
                exs[b] = ex

            def sm2a(b):
                denE = ppsm.tile([128, 512], F32, tag="psm", name="dE1")
                nc.tensor.matmul(denE[:T2, :64], M25, exs[b][:])
                rdenE = wk.tile([T2, 64], BF, tag="rden1")
                nc.vector.reciprocal(rdenE[:], denE[:T2, :64])
                alpha = wk.tile([T2, 64], BF, tag="al1")
                nc.vector.tensor_mul(alpha[:], exs[b][:], rdenE[:])
                astr = wk.tile([T2, CPB * 10], BF, tag="as1")
                av = alpha[:].rearrange("t (hf r h) -> t hf r h", hf=2, h=2)
                s4 = stair2.rearrange("t (g h) -> t g h", h=2)
                s4 = s4.unsqueeze(2).broadcast_to([T2, G2, 16, 2])
                ov = astr[:].rearrange("t (hf g r h) -> t hf g r h",
                                       g=G2, r=16, h=2)
                for hf in range(2):
                    a4 = av[:, hf, :, :].unsqueeze(1).broadcast_to(
                        [T2, G2, 16, 2])
                    nc.vector.tensor_mul(ov[:, hf, :, :, :], a4, s4)
                astrs[b] = astr

            def sm2c(b):
                hb = pieces[b]
                astr = astrs[b]
                for half in range(2):
                    q = 2 * b + half
                    pagg = ppagg.tile([128, 256], F32, tag="pagg",
                                      name="pagg1")
                    for rr in range(16):
                        cl = 16 * half + rr
                        for g in range(G2):
                            c0 = 160 * half + 32 * g + 2 * rr
                            nc.tensor.matmul(pagg[:, 32 * g + 2 * rr:
                                                  32 * g + 2 * rr + 2],
                                             hb[:, cl * FD:(cl + 1) * FD],
                                             astr[:, c0:c0 + 2])
                    nc.scalar.copy(aggT1[:, 160 * q:160 * (q + 1)],
                                   pagg[:, :160])

            def stage_fl(b):
                """aggRow transposes + en_L1 for block b's two flushes."""
                mv = mview()
                for half in range(2):
                    q = 2 * b + half
                    pagr = ppt.tile([128, 1024], BF, tag="pt", name="pagr")
                    sls = [mv[:, q, h, :] for h in range(H)]
                    for h in range(H):
                        nc.tensor.transpose(pagr[:T1, 128 * h:128 * (h + 1)],
                                            sls[h], ident[:])
                    for h in range(H):
                        nc.tensor.matmul(penL[:, 2 * q:2 * q + 2], sls[h],
                                         S["Wv1"][:, 2 * h:2 * h + 2],
                                         start=False, stop=(h == 1),
                                         skip_group_check=True)
                    if half:
                        nc.scalar.copy(aggRow[:, 256 * q:256 * (q + 1)],
                                       pagr[:T1, :256])
                    else:
                        nc.vector.tensor_copy(
                            aggRow[:, 256 * q:256 * (q + 1)],
                            pagr[:T1, :256])

            # ---- emission ----
            for it in range(NBLK + 2):
                cur = it if it < NBLK else None
                prv = it - 1 if 1 <= it <= NBLK else None
                if prv is not None:
                    sm2a(prv)
                    sm2c(prv)
                if cur is not None:
                    stage1_en(cur)
                    stage1_sm1(cur)
                if it >= 2:
                    stage_fl(it - 2)

            # ============ L1 tail ============
            astrL = softmax_T1(penL[:T1, :32], "L1")
            pr2 = []
            for h in range(H):
                p = ppagg.tile([128, 256], F32, tag="pagg", name=f"pre2_{h}")
                pr2.append(p)
            for cc in range(NC1):
                for h in range(H):
                    nc.tensor.matmul(
                        pr2[h][:, 16 * cc:16 * cc + 16],
                        aggRow[:, 256 * cc + 128 * h:256 * cc + 128 * (h + 1)],
                        astrL[:, 16 * cc:16 * cc + 16])
            nc.vector.tensor_copy(pre2sb[:, :256], pr2[0][:, :256])
            nc.scalar.copy(pre2sb[:, 256:], pr2[1][:, :256])
            py = ppt.tile([128, 128], F32, tag="pt", name="py")
            p2v = pre2sb[:].rearrange("p (h m hp) -> p h hp m",
                                      h=2, hp=2)
            idx = 0
            for hp in range(H):
                for h in range(H):
                    nc.tensor.matmul(py[:, :128], p2v[:, h, hp, :],
                                     S["H4"][:, (2 * hp + h) * 128:
                                             (2 * hp + h + 1) * 128],
                                     start=(idx == 0), stop=(idx == 3))
                    idx += 1
            nc.vector.tensor_copy(ysb[:], py[:])
            nc.sync.dma_start(yd[:], ysb[:])

    if split_waits:
        _split_multi_waits(nc)
    return nc


_PROG = None


def kernel(**inputs):
    global _PROG
    _install_drain_patch()
    prm = host_params(inputs["W0"], inputs["a_self0"], inputs["a_neigh0"],
                      inputs["W1"], inputs["a_self1"], inputs["a_neigh1"],
                      inputs["fc_W"])
    h0pk, h1pk, h2pk, h2T = pack_inputs(inputs["h0"], inputs["h1"],
                                        inputs["h2"])
    if _PROG is None:
        _PROG = build_program()
    nc = _PROG
    in_maps = []
    for c in range(NCORES):
        in_maps.append({"h0pk": h0pk[c], "h1pk": h1pk[c], "h2pk": h2pk[c],
                        "h2T": h2T[c], "prm": prm})
    core_ids = list(range(NCORES))
    last = None
    for _attempt in range(3):
        try:
            res = run_bass_kernel_spmd(nc, in_maps, core_ids)
            out = np.concatenate([np.asarray(res.results[c]["y"])
                                  for c in core_ids], axis=0)
            return out.astype(np.float32)
        except Exception as e:   # transient device-unrecoverable happens
            last = e
    raise last
